# revision 3
# baseline (speedup 1.0000x reference)
"""GAT layer kernel (v6) for Trainium2 (Bass/Tile), data-parallel over batch on 8 cores.

v5: col-tiled concurrent mask matvecs (tile_position=(0,32j)); W2-reduction
matmuls replace transpose tails; ACT Prelu for lrelu; fp32 x-transpose then
split; 2-block input DMAs; PE warmup stream; +-1/2 masks with total-sum
corrections.

Per-core computation (batch b, N=2048, F=128):
    s = x @ (W @ w_mlp) + b;  p = exp(s), q = exp(0.2 s)
    mt[i,j] = [s_i + s_j > 0] - 1/2   (symmetric, values +-1/2)
    D_i = p_i ((mt p)_i + Ptot/2) + q_i ((mt (-q))_i + Qtot/2)
    col  = p ((mt r) + Rtot/2) + q ((mt (-u)) + Utot/2),  r = p/D, u = q/D
    out  = lrelu(h) * col,  h = x @ W
"""

import sys

if "/opt/trn_rl_repo" not in sys.path:
    sys.path.insert(0, "/opt/trn_rl_repo")

from contextlib import ExitStack

import numpy as np

import concourse.bass as bass
import concourse.mybir as mybir
import concourse.tile as tile
from concourse import bacc
from concourse import masks
from concourse.bass_utils import run_bass_kernel_spmd

B, N, F = 8, 2048, 128
NB = N // 128  # 16 token blocks
NC4 = 4  # 512-wide chunks
NEG_SLOPE = 0.2
FP32 = mybir.dt.float32
BF16 = mybir.dt.bfloat16
ALU = mybir.AluOpType
AFT = mybir.ActivationFunctionType

# mask block -> engine: "v" DVE (is_gt - 0.5), "a" ACT (Sign, +-1, halved
# stationary)
MASK_ENG = {a: "v" for a in range(NB)}
for a in (4, 9, 14):
    MASK_ENG[a] = "a"
# mv1 consumption order ~ mask readiness; last four = one per col-group,
# consumed c-outer so d-chunk tails pipeline.
MV1_ORDER = [0, 1, 2, 3, 5, 6, 7, 8, 4, 10, 11, 9]
MV1_LAST = [12, 13, 15, 14]
# ACT Prelu not implemented in CoreSim; sim_check flips this off.
USE_PRELU = True


def gat_kernel(ctx: ExitStack, tc: "tile.TileContext", out_d, x_d, W_d, wm_d, bm_d):
    nc = tc.nc

    const_p = ctx.enter_context(tc.tile_pool(name="const", bufs=1))
    big_p = ctx.enter_context(tc.tile_pool(name="big", bufs=1))
    mask_p = ctx.enter_context(tc.tile_pool(name="mask", bufs=NB))
    vec_p = ctx.enter_context(tc.tile_pool(name="vec", bufs=1))
    outsb_p = ctx.enter_context(tc.tile_pool(name="outsb", bufs=4))
    ps_big = ctx.enter_context(tc.tile_pool(name="ps_big", bufs=1, space="PSUM"))
    ps_tr = ctx.enter_context(tc.tile_pool(name="ps_tr", bufs=3, space="PSUM"))
    ps_sm = ctx.enter_context(tc.tile_pool(name="ps_sm", bufs=1, space="PSUM"))

    dma_eng = [nc.sync, nc.scalar, nc.gpsimd]

    # ---------------- input DMAs first (x is the critical path) ----------
    W_sb = const_p.tile([128, 128], FP32, tag="W_sb")
    nc.sync.dma_start(W_sb[:], W_d[:, :])
    wm_sb = const_p.tile([128, 1], FP32, tag="wm_sb")
    nc.scalar.dma_start(wm_sb[:], wm_d.rearrange("(p o) -> p o", o=1))
    b_sb = const_p.tile([1, 1], FP32, tag="b_sb")
    nc.scalar.dma_start(b_sb[:], bm_d.rearrange("(p o) -> p o", o=1))
    x_view = x_d.rearrange("(t p) f -> p t f", p=128)
    x_sb = big_p.tile([128, NB, 128], FP32, tag="x_sb")
    for h in range(NB // 2):
        dma_eng[h % 3].dma_start(
            x_sb[:, 2 * h : 2 * h + 2, :], x_view[:, 2 * h : 2 * h + 2, :]
        )

    # ---------------- constants ----------------
    ident_f = const_p.tile([128, 128], FP32, tag="ident_f")
    ident_b = const_p.tile([128, 128], BF16, tag="ident_b")
    masks.make_identity(nc, ident_f[:])
    masks.make_identity(nc, ident_b[:])
    ones_f = const_p.tile([128, 1], FP32, tag="ones_f")
    nc.gpsimd.memset(ones_f[:], 1.0)
    ones_row_f = const_p.tile([1, 128], FP32, tag="ones_row_f")
    nc.gpsimd.memset(ones_row_f[:], 1.0)
    halves_row_f = const_p.tile([1, 128], FP32, tag="halves_row_f")
    nc.gpsimd.memset(halves_row_f[:], 0.5)
    ones_row_b = const_p.tile([1, 128], BF16, tag="ones_row_b")
    nc.gpsimd.memset(ones_row_b[:], 1.0)
    # zero-padded 32-col stationaries (cols 4..31 stay 0 so every col-group
    # writes its full 32-partition PSUM range)
    Pk = vec_p.tile([128, NB, 32], BF16, tag="Pk")
    Pkh = vec_p.tile([128, NB, 32], BF16, tag="Pkh")
    Rk = vec_p.tile([128, NB, 32], BF16, tag="Rk")
    Rkh = vec_p.tile([128, NB, 32], BF16, tag="Rkh")
    for stat in (Pk, Pkh, Rk, Rkh):
        nc.gpsimd.memset(stat[:], 0.0)

    # W2 [128, 2]: col0 selects rows {32j+0,32j+1}, col1 rows {32j+2,32j+3}
    W2 = const_p.tile([128, 2], FP32, tag="W2")
    ioti = const_p.tile([128, 1], mybir.dt.int32, tag="ioti")
    nc.gpsimd.iota(ioti[:], [[0, 1]], base=0, channel_multiplier=1)
    m32i = const_p.tile([128, 1], mybir.dt.int32, tag="m32i")
    nc.vector.tensor_scalar(m32i[:], ioti[:], 31, None, ALU.bitwise_and)
    m32 = const_p.tile([128, 1], FP32, tag="m32")
    nc.vector.tensor_copy(m32[:], m32i[:])
    nc.vector.tensor_scalar(W2[:, 0:1], m32[:], 2.0, None, ALU.is_lt)
    lt4 = const_p.tile([128, 1], FP32, tag="lt4")
    nc.vector.tensor_scalar(lt4[:], m32[:], 4.0, None, ALU.is_lt)
    nc.vector.tensor_tensor(W2[:, 1:2], lt4[:], W2[:, 0:1], ALU.subtract)

    # Preload the ACT table set early (exp_and_others also holds sign, copy,
    # parametric_relu)
    warm = const_p.tile([128, 2], FP32, tag="warm")
    nc.scalar.activation(warm[:, 0:1], ones_f[:], AFT.Exp)
    nc.scalar.activation(warm[:, 1:2], ones_f[:], AFT.Sign)

    # PE warmup stream: dummy fp32 matmuls on W/x block 0 (both DMA'd first)
    # so HAM reaches 2.4 GHz before the transposes (runs during the x DMA wait)
    warm_ps = ps_big.tile([128, 128], FP32, tag="bigps")
    for w in range(6):
        nc.tensor.matmul(
            warm_ps[:], lhsT=W_sb[:], rhs=x_sb[:, 0, :],
            start=(w == 0), stop=(w == 5),
        )

    # b broadcast to [128,1] via K=1 PE matmul
    b_ps = ps_sm.tile([128, 1], FP32, tag="sm")
    nc.tensor.matmul(b_ps[:], lhsT=ones_row_f[:], rhs=b_sb[:], start=True, stop=True)
    b_bc = const_p.tile([128, 1], FP32, tag="b_bc")
    nc.vector.tensor_copy(b_bc[:], b_ps[:])

    # ---------------- v = W @ w_mlp (via W^T), bf16 pair vk ----------------
    WT_ps = ps_sm.tile([128, 128], FP32, tag="sm")
    nc.tensor.transpose(WT_ps[:], W_sb[:], ident_f[:])
    WT_sb = vec_p.tile([128, 128], FP32, tag="WT_sb")
    nc.vector.tensor_copy(WT_sb[:], WT_ps[:])
    v_ps = ps_sm.tile([128, 1], FP32, tag="sm")
    nc.tensor.matmul(v_ps[:], lhsT=WT_sb[:], rhs=wm_sb[:], start=True, stop=True)
    v_sb = vec_p.tile([128, 1], FP32, tag="v_sb")
    nc.vector.tensor_copy(v_sb[:], v_ps[:])
    vk = vec_p.tile([128, 2], BF16, tag="vk")
    nc.vector.tensor_copy(vk[:, 0:1], v_sb[:])
    v_hi32 = vec_p.tile([128, 1], FP32, tag="v_hi32")
    nc.vector.tensor_copy(v_hi32[:], vk[:, 0:1])
    nc.vector.tensor_tensor(vk[:, 1:2], v_sb[:], v_hi32[:], ALU.subtract)

    # ---------------- xT via fp32 PE transposes, then bf16 hi/lo split ----
    xT_f = big_p.tile([128, N], FP32, tag="xT_f")  # [f, tok]
    for t2 in range(NB // 2):
        tp = ps_tr.tile([128, 256], FP32, tag="tr")
        nc.tensor.matmul(
            tp[:, 0:128], lhsT=x_sb[:, 2 * t2, :], rhs=ident_f[:],
            is_transpose=True, start=True, stop=False,
        )
        nc.tensor.matmul(
            tp[:, 128:256], lhsT=x_sb[:, 2 * t2 + 1, :], rhs=ident_f[:],
            is_transpose=True, start=False, stop=True,
        )
        if t2 % 2 == 0:
            nc.vector.tensor_copy(xT_f[:, t2 * 256 : (t2 + 1) * 256], tp[:])
        else:
            nc.scalar.copy(xT_f[:, t2 * 256 : (t2 + 1) * 256], tp[:])
    xT_hi = big_p.tile([128, N], BF16, tag="xT_hi")
    xT_lo = big_p.tile([128, N], BF16, tag="xT_lo")
    for c in range(NC4):
        sl = slice(c * 512, (c + 1) * 512)
        nc.vector.tensor_copy(xT_hi[:, sl], xT_f[:, sl])
        nc.vector.scalar_tensor_tensor(
            xT_lo[:, sl], xT_hi[:, sl], -1.0, xT_f[:, sl], ALU.mult, ALU.add
        )

    # ---------------- s in [128, 16] layout from xT pair ----------------
    s4_ps = ps_sm.tile([128, NB, 3], FP32, tag="sm")
    for t in range(NB):
        sl = slice(t * 128, (t + 1) * 128)
        nc.tensor.matmul(
            s4_ps[:, t, 0:2], lhsT=xT_hi[:, sl], rhs=vk[:], start=True, stop=True
        )
        nc.tensor.matmul(
            s4_ps[:, t, 2:3], lhsT=xT_lo[:, sl], rhs=vk[:, 0:1], start=True, stop=True
        )
    s4_sb = vec_p.tile([128, NB, 3], FP32, tag="s4_sb")
    nc.vector.tensor_copy(s4_sb[:], s4_ps[:])
    s12 = vec_p.tile([128, NB], FP32, tag="s12")
    nc.vector.tensor_tensor(s12[:], s4_sb[:, :, 0], s4_sb[:, :, 1], ALU.add)
    s_mat = vec_p.tile([128, NB], FP32, tag="s_mat")
    nc.vector.tensor_tensor(s_mat[:], s12[:], s4_sb[:, :, 2], ALU.add)
    nc.vector.tensor_scalar(s_mat[:], s_mat[:], b_bc[:, 0:1], None, ALU.add)
    s_hi = vec_p.tile([128, NB], BF16, tag="s_hi")
    nc.vector.tensor_copy(s_hi[:], s_mat[:])

    # S_row broadcast first (critical path for masks): sT transpose, then 16
    # direct K=1 broadcast matmuls (no DMA hop)
    sT_ps = ps_sm.tile([16, 128], BF16, tag="sm")
    nc.tensor.transpose(sT_ps[:], s_hi[:], ident_b[:])
    sT_sb = vec_p.tile([16, 128], BF16, tag="sT_sb")
    nc.vector.tensor_copy(sT_sb[:], sT_ps[:])
    s_flat = vec_p.tile([1, N], BF16, tag="s_flat")
    nc.sync.dma_start(s_flat[0:1, :], sT_sb[:, :])
    S_row = big_p.tile([128, N], BF16, tag="S_row")
    for c in range(NC4):
        sl = slice(c * 512, (c + 1) * 512)
        S_ps = ps_tr.tile([128, 512], FP32, tag="tr")
        nc.tensor.matmul(
            S_ps[:], lhsT=ones_row_b[:], rhs=s_flat[0:1, sl], start=True, stop=True
        )
        if c % 2 == 0:
            nc.vector.tensor_copy(S_row[:, sl], S_ps[:])
        else:
            nc.scalar.copy(S_row[:, sl], S_ps[:])

    neg_s = vec_p.tile([128, NB], FP32, tag="neg_s")
    nc.vector.tensor_scalar(neg_s[:], s_mat[:], -1.0, None, ALU.mult)

    # p = exp(s), q = exp(0.2 s), bf16 hi/lo packed stationary Pk
    p_v = vec_p.tile([128, NB], FP32, tag="p_v")
    nc.scalar.activation(p_v[:], s_mat[:], AFT.Exp)
    q_v = vec_p.tile([128, NB], FP32, tag="q_v")
    nc.scalar.activation(q_v[:], s_mat[:], AFT.Exp, scale=NEG_SLOPE)

    nc.vector.tensor_copy(Pk[:, :, 0], p_v[:])
    p_hi32 = vec_p.tile([128, NB], FP32, tag="p_hi32")
    nc.vector.tensor_copy(p_hi32[:], Pk[:, :, 0])
    nc.vector.tensor_tensor(Pk[:, :, 1], p_v[:], p_hi32[:], ALU.subtract)
    nc.vector.tensor_scalar(Pk[:, :, 2], q_v[:], -1.0, None, ALU.mult)
    qn_hi32 = vec_p.tile([128, NB], FP32, tag="qn_hi32")
    nc.vector.tensor_copy(qn_hi32[:], Pk[:, :, 2])
    nc.vector.scalar_tensor_tensor(
        Pk[:, :, 3], qn_hi32[:], -1.0, q_v[:], ALU.mult, ALU.subtract
    )
    nc.vector.tensor_scalar(Pkh[:, :, 0:4], Pk[:, :, 0:4], 0.5, None, ALU.mult)

    # Ptot/Qtot -> half-total broadcast htot [128, 2] = [Ptot/2, Qtot/2]
    pq_s = vec_p.tile([128, 2], FP32, tag="pq_s")
    nc.vector.reduce_sum(pq_s[:, 0:1], p_v[:], axis=mybir.AxisListType.X)
    nc.vector.reduce_sum(pq_s[:, 1:2], q_v[:], axis=mybir.AxisListType.X)
    tot_ps = ps_sm.tile([1, 2], FP32, tag="sm")
    nc.tensor.matmul(tot_ps[:], lhsT=ones_f[:], rhs=pq_s[:], start=True, stop=True)
    tot_sb = vec_p.tile([1, 2], FP32, tag="tot_sb")
    nc.vector.tensor_copy(tot_sb[:], tot_ps[:])
    htot_ps = ps_sm.tile([128, 2], FP32, tag="sm")
    nc.tensor.matmul(
        htot_ps[:], lhsT=halves_row_f[:], rhs=tot_sb[:], start=True, stop=True
    )
    htot = vec_p.tile([128, 2], FP32, tag="htot")
    nc.vector.tensor_copy(htot[:], htot_ps[:])

    # ---------------- hT (bf16 pairs) + Prelu -> lrlT ----------------------
    W_hi = const_p.tile([128, 128], BF16, tag="W_hi")
    nc.scalar.copy(W_hi[:], W_sb[:])
    W_hi32 = const_p.tile([128, 128], FP32, tag="W_hi32")
    nc.scalar.copy(W_hi32[:], W_hi[:])
    W_lo = const_p.tile([128, 128], BF16, tag="W_lo")
    nc.vector.tensor_tensor(W_lo[:], W_sb[:], W_hi32[:], ALU.subtract)

    hT_ps = ps_big.tile([128, N], FP32, tag="bigps")
    for c in range(NC4):
        sl = slice(c * 512, (c + 1) * 512)
        nc.tensor.matmul(
            hT_ps[:, sl], lhsT=W_hi[:], rhs=xT_hi[:, sl], start=True, stop=False
        )
        nc.tensor.matmul(
            hT_ps[:, sl], lhsT=W_hi[:], rhs=xT_lo[:, sl], start=False, stop=False
        )
        nc.tensor.matmul(
            hT_ps[:, sl], lhsT=W_lo[:], rhs=xT_hi[:, sl], start=False, stop=True
        )
    lrlT = big_p.tile([128, N], FP32, tag="lrlT")
    for c in range(NC4):
        sl = slice(c * 512, (c + 1) * 512)
        if USE_PRELU:
            nc.scalar.activation(lrlT[:, sl], hT_ps[:, sl], AFT.Prelu, alpha=NEG_SLOPE)
        else:
            tmp = vec_p.tile([128, 512], FP32, tag="hTtmp")
            nc.vector.tensor_copy(tmp[:], hT_ps[:, sl])
            nc.vector.scalar_tensor_tensor(
                lrlT[:, sl], tmp[:], NEG_SLOPE, tmp[:], ALU.mult, ALU.max
            )

    # ---------------- masks mt (bf16 [128, 2048] per token block) ---------
    mask_tiles = []
    for a in range(NB):
        m = mask_p.tile([128, N], BF16, tag="mask")
        if MASK_ENG[a] == "v":
            nc.vector.tensor_scalar(
                m[:], S_row[:], neg_s[:, a : a + 1], 0.5, ALU.is_gt, ALU.subtract
            )
        else:
            nc.scalar.activation(m[:], S_row[:], AFT.Sign, bias=s_mat[:, a : a + 1])
        mask_tiles.append(m)

    def stat_for(base, base_h, a):
        return (base_h if MASK_ENG[a] == "a" else base)[:, a, :]

    first_of_group = {}
    for a in MV1_ORDER + MV1_LAST:
        j = a % 4
        if j not in first_of_group:
            first_of_group[j] = a

    # ---------------- matvec 1 (col-tiled; last round c-outer) ------------
    d_ps = ps_big.tile([128, N], FP32, tag="bigps")

    def mv1_mm(a, c, stop):
        j = a % 4
        nc.tensor.matmul(
            d_ps[32 * j : 32 * j + 32, c * 512 : (c + 1) * 512],
            lhsT=stat_for(Pk, Pkh, a),
            rhs=mask_tiles[a][:, c * 512 : (c + 1) * 512],
            start=(a == first_of_group[j]),
            stop=stop,
            tile_position=(0, 32 * j),
            skip_group_check=True,
        )

    for a in MV1_ORDER:
        for c in range(NC4):
            mv1_mm(a, c, False)

    # per-chunk tail: copy psum chunk to SBUF, then one K=128 reduction
    # matmul per token block: csb_block^T @ W2 -> [128 tok, 2] partials
    Dt2_ps = ps_sm.tile([128, NB, 2], FP32, tag="sm")

    def mv_tail_chunk(src_ps, dst2_ps, c, copy1):
        csb = vec_p.tile([128, 512], FP32, tag=f"csb{c % 2}")
        copy1(csb[:], src_ps[:, c * 512 : (c + 1) * 512])
        for tt in range(4):
            t = c * 4 + tt
            nc.tensor.matmul(
                dst2_ps[:, t, :],
                lhsT=csb[:, tt * 128 : (tt + 1) * 128],
                rhs=W2[:],
                start=True,
                stop=True,
            )

    for c in range(NC4):
        for j in range(4):
            mv1_mm(MV1_LAST[j], c, True)
        mv_tail_chunk(
            d_ps, Dt2_ps, c,
            nc.vector.tensor_copy if c % 2 == 0 else nc.scalar.copy,
        )

    def warm_mm(i):
        wp = ps_tr.tile([128, 512], FP32, tag="tr")
        nc.tensor.matmul(
            wp[:], lhsT=ident_b[:], rhs=mask_tiles[i % NB][:, 0:512],
            start=True, stop=True,
        )

    Dt2 = vec_p.tile([128, NB, 2], FP32, tag="Dt2")
    nc.vector.tensor_copy(Dt2[:], Dt2_ps[:])

    # ---------------- combine: A/B [128, NB], D, Rk ------------------------
    A_v = vec_p.tile([128, NB], FP32, tag="A_v")
    nc.vector.tensor_scalar(A_v[:], Dt2[:, :, 0], htot[:, 0:1], None, ALU.add)
    B_v = vec_p.tile([128, NB], FP32, tag="B_v")
    nc.vector.tensor_scalar(B_v[:], Dt2[:, :, 1], htot[:, 1:2], None, ALU.add)
    t1 = vec_p.tile([128, NB], FP32, tag="t1")
    nc.vector.tensor_tensor(t1[:], p_v[:], A_v[:], ALU.mult)
    t2 = vec_p.tile([128, NB], FP32, tag="t2")
    nc.vector.tensor_tensor(t2[:], q_v[:], B_v[:], ALU.mult)
    D_v = vec_p.tile([128, NB], FP32, tag="D_v")
    nc.vector.tensor_tensor(D_v[:], t1[:], t2[:], ALU.add)
    warm_mm(4)
    invD = vec_p.tile([128, NB], FP32, tag="invD")
    nc.vector.reciprocal(invD[:], D_v[:])
    r_v = vec_p.tile([128, NB], FP32, tag="r_v")
    nc.vector.tensor_tensor(r_v[:], p_v[:], invD[:], ALU.mult)
    u_v = vec_p.tile([128, NB], FP32, tag="u_v")
    nc.vector.tensor_tensor(u_v[:], q_v[:], invD[:], ALU.mult)
    nc.vector.tensor_copy(Rk[:, :, 0], r_v[:])
    r_hi32 = vec_p.tile([128, NB], FP32, tag="r_hi32")
    nc.vector.tensor_copy(r_hi32[:], Rk[:, :, 0])
    nc.vector.tensor_tensor(Rk[:, :, 1], r_v[:], r_hi32[:], ALU.subtract)
    nc.vector.tensor_scalar(Rk[:, :, 2], u_v[:], -1.0, None, ALU.mult)
    un_hi32 = vec_p.tile([128, NB], FP32, tag="un_hi32")
    nc.vector.tensor_copy(un_hi32[:], Rk[:, :, 2])
    nc.vector.scalar_tensor_tensor(
        Rk[:, :, 3], un_hi32[:], -1.0, u_v[:], ALU.mult, ALU.subtract
    )
    nc.vector.tensor_scalar(Rkh[:, :, 0:4], Rk[:, :, 0:4], 0.5, None, ALU.mult)
    warm_mm(5)

    # Rtot/Utot -> half-total broadcast htot2 [128, 2]
    ru_s = vec_p.tile([128, 2], FP32, tag="ru_s")
    nc.vector.reduce_sum(ru_s[:, 0:1], r_v[:], axis=mybir.AxisListType.X)
    nc.vector.reduce_sum(ru_s[:, 1:2], u_v[:], axis=mybir.AxisListType.X)
    tot2_ps = ps_sm.tile([1, 2], FP32, tag="sm")
    nc.tensor.matmul(tot2_ps[:], lhsT=ones_f[:], rhs=ru_s[:], start=True, stop=True)
    tot2_sb = vec_p.tile([1, 2], FP32, tag="tot2_sb")
    nc.vector.tensor_copy(tot2_sb[:], tot2_ps[:])
    htot2_ps = ps_sm.tile([128, 2], FP32, tag="sm")
    nc.tensor.matmul(
        htot2_ps[:], lhsT=halves_row_f[:], rhs=tot2_sb[:], start=True, stop=True
    )
    htot2 = vec_p.tile([128, 2], FP32, tag="htot2")
    nc.vector.tensor_copy(htot2[:], htot2_ps[:])

    # ---------------- lrl transposes [tok, f], just-in-time per chunk -----
    lrl_sb = big_p.tile([128, NB, 128], FP32, tag="lrl_sb")

    def lrl_chunk(c):
        for h in range(2):
            t2 = 2 * c + h
            op = ps_tr.tile([128, 256], FP32, tag="tr")
            nc.tensor.matmul(
                op[:, 0:128], lhsT=lrlT[:, 256 * t2 : 256 * t2 + 128], rhs=ident_f[:],
                is_transpose=True, start=True, stop=False,
            )
            nc.tensor.matmul(
                op[:, 128:256], lhsT=lrlT[:, 256 * t2 + 128 : 256 * t2 + 256],
                rhs=ident_f[:],
                is_transpose=True, start=False, stop=True,
            )
            if h == 0:
                nc.scalar.copy(lrl_sb[:, 2 * t2 : 2 * t2 + 2, :], op[:])
            else:
                nc.vector.tensor_copy(lrl_sb[:, 2 * t2 : 2 * t2 + 2, :], op[:])

    for _c in range(NC4):
        lrl_chunk(_c)
        if _c % 2 == 1:
            warm_mm(6 + _c)

    # ---------------- matvec 2 (c-outer, col-tiled, pipelined tails) ------
    out_view = out_d.rearrange("(t p) f -> p t f", p=128)
    g_ps = ps_big.tile([128, N], FP32, tag="bigps")
    Gt2_ps = ps_sm.tile([128, NB, 2], FP32, tag="sm")
    gsb = vec_p.tile([128, NB, 2], FP32, tag="gsb")
    gA = vec_p.tile([128, NB], FP32, tag="gA")
    gB = vec_p.tile([128, NB], FP32, tag="gB")
    gt1 = vec_p.tile([128, NB], FP32, tag="gt1")
    col = vec_p.tile([128, NB], FP32, tag="col")

    def mv2_chunk(c):
        for rr in range(NC4):
            for j in range(4):
                a = 4 * rr + j
                nc.tensor.matmul(
                    g_ps[32 * j : 32 * j + 32, c * 512 : (c + 1) * 512],
                    lhsT=stat_for(Rk, Rkh, a),
                    rhs=mask_tiles[a][:, c * 512 : (c + 1) * 512],
                    start=(rr == 0),
                    stop=(rr == 3),
                    tile_position=(0, 32 * j),
                    skip_group_check=True,
                )

    def mv2_tail(c):
        mv_tail_chunk(g_ps, Gt2_ps, c, nc.scalar.copy)
        ts = slice(c * 4, (c + 1) * 4)
        nc.vector.tensor_copy(gsb[:, ts, :], Gt2_ps[:, ts, :])
        nc.vector.tensor_scalar(gA[:, ts], gsb[:, ts, 0], htot2[:, 0:1], None, ALU.add)
        nc.vector.tensor_scalar(gB[:, ts], gsb[:, ts, 1], htot2[:, 1:2], None, ALU.add)
        nc.vector.tensor_tensor(gt1[:, ts], p_v[:, ts], gA[:, ts], ALU.mult)
        nc.vector.tensor_tensor(col[:, ts], q_v[:, ts], gB[:, ts], ALU.mult)
        nc.vector.tensor_tensor(col[:, ts], col[:, ts], gt1[:, ts], ALU.add)
        for tt in range(4):
            t = c * 4 + tt
            o_sb = outsb_p.tile([128, 128], FP32, tag="o_sb")
            nc.vector.tensor_scalar(
                o_sb[:], lrl_sb[:, t, :], col[:, t : t + 1], None, ALU.mult
            )
            dma_eng[0 if tt % 2 == 0 else 2].dma_start(out_view[:, t, :], o_sb[:])

    mv2_chunk(0)
    mv2_chunk(1)
    mv2_tail(0)
    mv2_chunk(2)
    mv2_tail(1)
    mv2_chunk(3)
    mv2_tail(2)
    mv2_tail(3)


def build_nc(num_devices: int = 8) -> "bass.Bass":
    nc = bacc.Bacc(
        "TRN2", target_bir_lowering=False, debug=False, num_devices=num_devices
    )
    x_d = nc.dram_tensor("x", [N, F], FP32, kind="ExternalInput")
    W_d = nc.dram_tensor("W", [F, F], FP32, kind="ExternalInput")
    wm_d = nc.dram_tensor("w_mlp", [F], FP32, kind="ExternalInput")
    bm_d = nc.dram_tensor("b_mlp", [1], FP32, kind="ExternalInput")
    out_d = nc.dram_tensor("out", [N, F], FP32, kind="ExternalOutput")
    with tile.TileContext(nc) as tc:
        with ExitStack() as ctx:
            gat_kernel(ctx, tc, out_d.ap(), x_d.ap(), W_d.ap(), wm_d.ap(), bm_d.ap())
    nc.compile()
    return nc


_NC_CACHE: dict = {}


def run(x, W, w_mlp, b_mlp, trace=False, **spmd_kwargs):
    x = np.asarray(x, dtype=np.float32)
    W = np.asarray(W, dtype=np.float32)
    w_mlp = np.asarray(w_mlp, dtype=np.float32)
    b_mlp = np.asarray(b_mlp, dtype=np.float32)

    if "nc" not in _NC_CACHE:
        _NC_CACHE["nc"] = build_nc(num_devices=B)
    nc = _NC_CACHE["nc"]

    in_maps = [
        {"x": np.ascontiguousarray(x[b, 0]), "W": W, "w_mlp": w_mlp, "b_mlp": b_mlp}
        for b in range(B)
    ]
    res = run_bass_kernel_spmd(
        nc, in_maps, core_ids=list(range(B)), trace=trace, **spmd_kwargs
    )
    out = np.stack([res.results[b]["out"] for b in range(B)])[:, None]
    return out.astype(np.float32), res


def kernel(x, W, w_mlp, b_mlp):
    out, _ = run(x, W, w_mlp, b_mlp)
    return out


# revision 5
# speedup vs baseline: 1.0410x; 1.0410x over previous
"""GAT layer kernel (v6) for Trainium2 (Bass/Tile), data-parallel over batch on 8 cores.

v5: col-tiled concurrent mask matvecs (tile_position=(0,32j)); W2-reduction
matmuls replace transpose tails; ACT Prelu for lrelu; fp32 x-transpose then
split; 2-block input DMAs; PE warmup stream; +-1/2 masks with total-sum
corrections.

Per-core computation (batch b, N=2048, F=128):
    s = x @ (W @ w_mlp) + b;  p = exp(s), q = exp(0.2 s)
    mt[i,j] = [s_i + s_j > 0] - 1/2   (symmetric, values +-1/2)
    D_i = p_i ((mt p)_i + Ptot/2) + q_i ((mt (-q))_i + Qtot/2)
    col  = p ((mt r) + Rtot/2) + q ((mt (-u)) + Utot/2),  r = p/D, u = q/D
    out  = lrelu(h) * col,  h = x @ W
"""

import sys

if "/opt/trn_rl_repo" not in sys.path:
    sys.path.insert(0, "/opt/trn_rl_repo")

from contextlib import ExitStack

import numpy as np

import concourse.bass as bass
import concourse.mybir as mybir
import concourse.tile as tile
from concourse import bacc
from concourse import masks
from concourse.bass_utils import run_bass_kernel_spmd

B, N, F = 8, 2048, 128
NB = N // 128  # 16 token blocks
NC4 = 4  # 512-wide chunks
NEG_SLOPE = 0.2
FP32 = mybir.dt.float32
BF16 = mybir.dt.bfloat16
ALU = mybir.AluOpType
AFT = mybir.ActivationFunctionType

# mask block -> engine: "v" DVE (is_gt - 0.5), "a" ACT (Sign, +-1, halved
# stationary)
MASK_ENG = {a: "v" for a in range(NB)}
for a in (4, 9, 14):
    MASK_ENG[a] = "a"
# mv1 consumption order ~ mask readiness; last four = one per col-group,
# consumed c-outer so d-chunk tails pipeline.
MV1_ORDER = [0, 1, 2, 3, 5, 6, 7, 8, 4, 10, 11, 9]
MV1_LAST = [12, 13, 15, 14]
# ACT Prelu not implemented in CoreSim; sim_check flips this off.
USE_PRELU = True


def gat_kernel(ctx: ExitStack, tc: "tile.TileContext", out_d, x_d, W_d, wm_d, bm_d):
    nc = tc.nc

    const_p = ctx.enter_context(tc.tile_pool(name="const", bufs=1))
    big_p = ctx.enter_context(tc.tile_pool(name="big", bufs=1))
    mask_p = ctx.enter_context(tc.tile_pool(name="mask", bufs=NB))
    vec_p = ctx.enter_context(tc.tile_pool(name="vec", bufs=1))
    outsb_p = ctx.enter_context(tc.tile_pool(name="outsb", bufs=4))
    ps_big = ctx.enter_context(tc.tile_pool(name="ps_big", bufs=1, space="PSUM"))
    ps_tr = ctx.enter_context(tc.tile_pool(name="ps_tr", bufs=3, space="PSUM"))
    ps_sm = ctx.enter_context(tc.tile_pool(name="ps_sm", bufs=1, space="PSUM"))

    dma_eng = [nc.sync, nc.scalar, nc.gpsimd]

    # ---------------- input DMAs first (x is the critical path) ----------
    W_sb = const_p.tile([128, 128], FP32, tag="W_sb")
    nc.sync.dma_start(W_sb[:], W_d[:, :])
    wm_sb = const_p.tile([128, 1], FP32, tag="wm_sb")
    nc.scalar.dma_start(wm_sb[:], wm_d.rearrange("(p o) -> p o", o=1))
    b_sb = const_p.tile([1, 1], FP32, tag="b_sb")
    nc.scalar.dma_start(b_sb[:], bm_d.rearrange("(p o) -> p o", o=1))
    # identities first on the gpsimd queue: they gate the first transposes
    ident_f = const_p.tile([128, 128], FP32, tag="ident_f")
    ident_b = const_p.tile([128, 128], BF16, tag="ident_b")
    masks.make_identity(nc, ident_f[:])
    masks.make_identity(nc, ident_b[:])
    x_view = x_d.rearrange("(t p) f -> p t f", p=128)
    x_sb = big_p.tile([128, NB, 128], FP32, tag="x_sb")
    for h in range(NB // 2):
        dma_eng[h % 3].dma_start(
            x_sb[:, 2 * h : 2 * h + 2, :], x_view[:, 2 * h : 2 * h + 2, :]
        )

    # ---------------- constants ----------------
    ones_f = const_p.tile([128, 1], FP32, tag="ones_f")
    nc.gpsimd.memset(ones_f[:], 1.0)
    ones_row_f = const_p.tile([1, 128], FP32, tag="ones_row_f")
    nc.gpsimd.memset(ones_row_f[:], 1.0)
    halves_row_f = const_p.tile([1, 128], FP32, tag="halves_row_f")
    nc.gpsimd.memset(halves_row_f[:], 0.5)
    ones_row_b = const_p.tile([1, 128], BF16, tag="ones_row_b")
    nc.gpsimd.memset(ones_row_b[:], 1.0)
    # zero-padded 32-col stationaries (cols 4..31 stay 0 so every col-group
    # writes its full 32-partition PSUM range)
    Pk = vec_p.tile([128, NB, 32], BF16, tag="Pk")
    Pkh = vec_p.tile([128, NB, 32], BF16, tag="Pkh")
    Rk = vec_p.tile([128, NB, 32], BF16, tag="Rk")
    Rkh = vec_p.tile([128, NB, 32], BF16, tag="Rkh")
    for stat in (Pk, Pkh, Rk, Rkh):
        nc.gpsimd.memset(stat[:], 0.0)

    # W2 [128, 2]: col0 selects rows {32j+0,32j+1}, col1 rows {32j+2,32j+3}
    W2 = const_p.tile([128, 2], FP32, tag="W2")
    ioti = const_p.tile([128, 1], mybir.dt.int32, tag="ioti")
    nc.gpsimd.iota(ioti[:], [[0, 1]], base=0, channel_multiplier=1)
    m32i = const_p.tile([128, 1], mybir.dt.int32, tag="m32i")
    nc.vector.tensor_scalar(m32i[:], ioti[:], 31, None, ALU.bitwise_and)
    m32 = const_p.tile([128, 1], FP32, tag="m32")
    nc.vector.tensor_copy(m32[:], m32i[:])
    nc.vector.tensor_scalar(W2[:, 0:1], m32[:], 2.0, None, ALU.is_lt)
    lt4 = const_p.tile([128, 1], FP32, tag="lt4")
    nc.vector.tensor_scalar(lt4[:], m32[:], 4.0, None, ALU.is_lt)
    nc.vector.tensor_tensor(W2[:, 1:2], lt4[:], W2[:, 0:1], ALU.subtract)

    # Preload the ACT table set early (exp_and_others also holds sign, copy,
    # parametric_relu)
    warm = const_p.tile([128, 2], FP32, tag="warm")
    nc.scalar.activation(warm[:, 0:1], ones_f[:], AFT.Exp)
    nc.scalar.activation(warm[:, 1:2], ones_f[:], AFT.Sign)

    # b broadcast to [128,1] via K=1 PE matmul
    b_ps = ps_sm.tile([128, 1], FP32, tag="sm")
    nc.tensor.matmul(b_ps[:], lhsT=ones_row_f[:], rhs=b_sb[:], start=True, stop=True)
    b_bc = const_p.tile([128, 1], FP32, tag="b_bc")
    nc.vector.tensor_copy(b_bc[:], b_ps[:])

    # ---------------- v = W @ w_mlp (via W^T), bf16 pair vk ----------------
    WT_ps = ps_sm.tile([128, 128], FP32, tag="sm")
    nc.tensor.transpose(WT_ps[:], W_sb[:], ident_f[:])
    WT_sb = vec_p.tile([128, 128], FP32, tag="WT_sb")
    nc.vector.tensor_copy(WT_sb[:], WT_ps[:])
    v_ps = ps_sm.tile([128, 1], FP32, tag="sm")
    nc.tensor.matmul(v_ps[:], lhsT=WT_sb[:], rhs=wm_sb[:], start=True, stop=True)
    v_sb = vec_p.tile([128, 1], FP32, tag="v_sb")
    nc.vector.tensor_copy(v_sb[:], v_ps[:])
    vk = vec_p.tile([128, 2], BF16, tag="vk")
    nc.vector.tensor_copy(vk[:, 0:1], v_sb[:])
    v_hi32 = vec_p.tile([128, 1], FP32, tag="v_hi32")
    nc.vector.tensor_copy(v_hi32[:], vk[:, 0:1])
    nc.vector.tensor_tensor(vk[:, 1:2], v_sb[:], v_hi32[:], ALU.subtract)

    # ---------------- xT via fp32 PE transposes, then bf16 hi/lo split ----
    xT_f = big_p.tile([128, N], FP32, tag="xT_f")  # [f, tok]
    for t2 in range(NB // 2):
        tp = ps_tr.tile([128, 256], FP32, tag="tr")
        nc.tensor.matmul(
            tp[:, 0:128], lhsT=x_sb[:, 2 * t2, :], rhs=ident_f[:],
            is_transpose=True, start=True, stop=False,
        )
        nc.tensor.matmul(
            tp[:, 128:256], lhsT=x_sb[:, 2 * t2 + 1, :], rhs=ident_f[:],
            is_transpose=True, start=False, stop=True,
        )
        if t2 % 2 == 0:
            nc.vector.tensor_copy(xT_f[:, t2 * 256 : (t2 + 1) * 256], tp[:])
        else:
            nc.scalar.copy(xT_f[:, t2 * 256 : (t2 + 1) * 256], tp[:])
    xT_hi = big_p.tile([128, N], BF16, tag="xT_hi")
    xT_lo = big_p.tile([128, N], BF16, tag="xT_lo")
    for c in range(NC4):
        sl = slice(c * 512, (c + 1) * 512)
        nc.vector.tensor_copy(xT_hi[:, sl], xT_f[:, sl])
        nc.vector.scalar_tensor_tensor(
            xT_lo[:, sl], xT_hi[:, sl], -1.0, xT_f[:, sl], ALU.mult, ALU.add
        )

    # ---------------- s in [128, 16] layout from xT pair ----------------
    s4_ps = ps_sm.tile([128, NB, 3], FP32, tag="sm")
    for t in range(NB):
        sl = slice(t * 128, (t + 1) * 128)
        nc.tensor.matmul(
            s4_ps[:, t, 0:2], lhsT=xT_hi[:, sl], rhs=vk[:], start=True, stop=True
        )
        nc.tensor.matmul(
            s4_ps[:, t, 2:3], lhsT=xT_lo[:, sl], rhs=vk[:, 0:1], start=True, stop=True
        )
    s4_sb = vec_p.tile([128, NB, 3], FP32, tag="s4_sb")
    s12 = vec_p.tile([128, NB], FP32, tag="s12")
    s_mat = vec_p.tile([128, NB], FP32, tag="s_mat")
    s_hi = vec_p.tile([128, NB], BF16, tag="s_hi")
    neg_s = vec_p.tile([128, NB], FP32, tag="neg_s")
    s_flat = vec_p.tile([1, N], BF16, tag="s_flat")
    S_row = big_p.tile([128, N], BF16, tag="S_row")
    # two-half s-chain: half h covers token blocks 8h..8h+7, so half-0's
    # S_row columns (and the left mask halves) are ready earlier
    for hh in range(2):
        bs = slice(hh * 8, (hh + 1) * 8)
        nc.vector.tensor_copy(s4_sb[:, bs, :], s4_ps[:, bs, :])
        nc.vector.tensor_tensor(s12[:, bs], s4_sb[:, bs, 0], s4_sb[:, bs, 1], ALU.add)
        nc.vector.tensor_tensor(s_mat[:, bs], s12[:, bs], s4_sb[:, bs, 2], ALU.add)
        nc.vector.tensor_scalar(s_mat[:, bs], s_mat[:, bs], b_bc[:, 0:1], None, ALU.add)
        nc.vector.tensor_copy(s_hi[:, bs], s_mat[:, bs])
        sT_ps = ps_sm.tile([8, 128], BF16, tag="sm")
        nc.tensor.transpose(sT_ps[:], s_hi[:, bs], ident_b[:])
        sT_sb = vec_p.tile([8, 128], BF16, tag=f"sT_sb{hh}")
        nc.vector.tensor_copy(sT_sb[:], sT_ps[:])
        nc.sync.dma_start(s_flat[0:1, hh * 1024 : (hh + 1) * 1024], sT_sb[:, :])
        for c in (2 * hh, 2 * hh + 1):
            sl = slice(c * 512, (c + 1) * 512)
            S_ps = ps_tr.tile([128, 512], FP32, tag="tr")
            nc.tensor.matmul(
                S_ps[:], lhsT=ones_row_b[:], rhs=s_flat[0:1, sl], start=True, stop=True
            )
            if c % 2 == 0:
                nc.vector.tensor_copy(S_row[:, sl], S_ps[:])
            else:
                nc.scalar.copy(S_row[:, sl], S_ps[:])
        nc.vector.tensor_scalar(neg_s[:, bs], s_mat[:, bs], -1.0, None, ALU.mult)

    # p = exp(s), q = exp(0.2 s), bf16 hi/lo packed stationary Pk
    p_v = vec_p.tile([128, NB], FP32, tag="p_v")
    nc.scalar.activation(p_v[:], s_mat[:], AFT.Exp)
    q_v = vec_p.tile([128, NB], FP32, tag="q_v")
    nc.scalar.activation(q_v[:], s_mat[:], AFT.Exp, scale=NEG_SLOPE)

    nc.vector.tensor_copy(Pk[:, :, 0], p_v[:])
    p_hi32 = vec_p.tile([128, NB], FP32, tag="p_hi32")
    nc.vector.tensor_copy(p_hi32[:], Pk[:, :, 0])
    nc.vector.tensor_tensor(Pk[:, :, 1], p_v[:], p_hi32[:], ALU.subtract)
    nc.vector.tensor_scalar(Pk[:, :, 2], q_v[:], -1.0, None, ALU.mult)
    qn_hi32 = vec_p.tile([128, NB], FP32, tag="qn_hi32")
    nc.vector.tensor_copy(qn_hi32[:], Pk[:, :, 2])
    nc.vector.scalar_tensor_tensor(
        Pk[:, :, 3], qn_hi32[:], -1.0, q_v[:], ALU.mult, ALU.subtract
    )
    nc.vector.tensor_scalar(Pkh[:, :, 0:4], Pk[:, :, 0:4], 0.5, None, ALU.mult)

    # Ptot/Qtot -> half-total broadcast htot [128, 2] = [Ptot/2, Qtot/2]
    pq_s = vec_p.tile([128, 2], FP32, tag="pq_s")
    nc.vector.reduce_sum(pq_s[:, 0:1], p_v[:], axis=mybir.AxisListType.X)
    nc.vector.reduce_sum(pq_s[:, 1:2], q_v[:], axis=mybir.AxisListType.X)
    tot_ps = ps_sm.tile([1, 2], FP32, tag="sm")
    nc.tensor.matmul(tot_ps[:], lhsT=ones_f[:], rhs=pq_s[:], start=True, stop=True)
    tot_sb = vec_p.tile([1, 2], FP32, tag="tot_sb")
    nc.vector.tensor_copy(tot_sb[:], tot_ps[:])
    htot_ps = ps_sm.tile([128, 2], FP32, tag="sm")
    nc.tensor.matmul(
        htot_ps[:], lhsT=halves_row_f[:], rhs=tot_sb[:], start=True, stop=True
    )
    htot = vec_p.tile([128, 2], FP32, tag="htot")
    nc.vector.tensor_copy(htot[:], htot_ps[:])

    # ---------------- hT (bf16 pairs) + Prelu -> lrlT ----------------------
    W_hi = const_p.tile([128, 128], BF16, tag="W_hi")
    nc.scalar.copy(W_hi[:], W_sb[:])
    W_hi32 = const_p.tile([128, 128], FP32, tag="W_hi32")
    nc.scalar.copy(W_hi32[:], W_hi[:])
    W_lo = const_p.tile([128, 128], BF16, tag="W_lo")
    nc.vector.tensor_tensor(W_lo[:], W_sb[:], W_hi32[:], ALU.subtract)

    hT_ps = ps_big.tile([128, N], FP32, tag="bigps")
    for c in range(NC4):
        sl = slice(c * 512, (c + 1) * 512)
        nc.tensor.matmul(
            hT_ps[:, sl], lhsT=W_hi[:], rhs=xT_hi[:, sl], start=True, stop=False
        )
        nc.tensor.matmul(
            hT_ps[:, sl], lhsT=W_hi[:], rhs=xT_lo[:, sl], start=False, stop=False
        )
        nc.tensor.matmul(
            hT_ps[:, sl], lhsT=W_lo[:], rhs=xT_hi[:, sl], start=False, stop=True
        )
    lrlT = big_p.tile([128, N], FP32, tag="lrlT")
    for c in range(NC4):
        sl = slice(c * 512, (c + 1) * 512)
        if USE_PRELU:
            nc.scalar.activation(lrlT[:, sl], hT_ps[:, sl], AFT.Prelu, alpha=NEG_SLOPE)
        else:
            tmp = vec_p.tile([128, 512], FP32, tag="hTtmp")
            nc.vector.tensor_copy(tmp[:], hT_ps[:, sl])
            nc.vector.scalar_tensor_tensor(
                lrlT[:, sl], tmp[:], NEG_SLOPE, tmp[:], ALU.mult, ALU.max
            )

    # ---------------- masks mt (bf16 [128, 2048] per token block) ---------
    mask_tiles = []
    for a in range(NB):
        m = mask_p.tile([128, N], BF16, tag="mask")
        if MASK_ENG[a] == "v":
            if a < 8:
                nc.vector.tensor_scalar(
                    m[:, 0:1024], S_row[:, 0:1024], neg_s[:, a : a + 1], 0.5,
                    ALU.is_gt, ALU.subtract,
                )
                nc.vector.tensor_scalar(
                    m[:, 1024:2048], S_row[:, 1024:2048], neg_s[:, a : a + 1], 0.5,
                    ALU.is_gt, ALU.subtract,
                )
            else:
                nc.vector.tensor_scalar(
                    m[:], S_row[:], neg_s[:, a : a + 1], 0.5, ALU.is_gt, ALU.subtract
                )
        else:
            nc.scalar.activation(m[:], S_row[:], AFT.Sign, bias=s_mat[:, a : a + 1])
        mask_tiles.append(m)

    def stat_for(base, base_h, a):
        return (base_h if MASK_ENG[a] == "a" else base)[:, a, :]

    first_of_group = {}
    for a in MV1_ORDER + MV1_LAST:
        j = a % 4
        if j not in first_of_group:
            first_of_group[j] = a

    # ---------------- matvec 1 (col-tiled; last round c-outer) ------------
    d_ps = ps_big.tile([128, N], FP32, tag="bigps")

    def mv1_mm(a, c, stop):
        j = a % 4
        nc.tensor.matmul(
            d_ps[32 * j : 32 * j + 32, c * 512 : (c + 1) * 512],
            lhsT=stat_for(Pk, Pkh, a),
            rhs=mask_tiles[a][:, c * 512 : (c + 1) * 512],
            start=(a == first_of_group[j]),
            stop=stop,
            tile_position=(0, 32 * j),
            skip_group_check=True,
        )

    for a in MV1_ORDER:
        for c in range(NC4):
            mv1_mm(a, c, False)

    # per-chunk tail: copy psum chunk to SBUF, then one K=128 reduction
    # matmul per token block: csb_block^T @ W2 -> [128 tok, 2] partials
    Dt2_ps = ps_sm.tile([128, NB, 2], FP32, tag="sm")

    def mv_tail_chunk(src_ps, dst2_ps, c, copy1):
        csb = vec_p.tile([128, 512], FP32, tag=f"csb{c % 2}")
        copy1(csb[:], src_ps[:, c * 512 : (c + 1) * 512])
        for tt in range(4):
            t = c * 4 + tt
            nc.tensor.matmul(
                dst2_ps[:, t, :],
                lhsT=csb[:, tt * 128 : (tt + 1) * 128],
                rhs=W2[:],
                start=True,
                stop=True,
            )

    for c in range(NC4):
        for j in range(4):
            mv1_mm(MV1_LAST[j], c, True)
        mv_tail_chunk(
            d_ps, Dt2_ps, c,
            nc.vector.tensor_copy if c % 2 == 0 else nc.scalar.copy,
        )

    def warm_mm(i):
        wp = ps_tr.tile([128, 512], FP32, tag="tr")
        nc.tensor.matmul(
            wp[:], lhsT=ident_b[:], rhs=mask_tiles[i % NB][:, 0:512],
            start=True, stop=True,
        )

    Dt2 = vec_p.tile([128, NB, 2], FP32, tag="Dt2")
    nc.vector.tensor_copy(Dt2[:], Dt2_ps[:])

    # ---------------- combine: A/B [128, NB], D, Rk ------------------------
    A_v = vec_p.tile([128, NB], FP32, tag="A_v")
    nc.vector.tensor_scalar(A_v[:], Dt2[:, :, 0], htot[:, 0:1], None, ALU.add)
    B_v = vec_p.tile([128, NB], FP32, tag="B_v")
    nc.vector.tensor_scalar(B_v[:], Dt2[:, :, 1], htot[:, 1:2], None, ALU.add)
    t1 = vec_p.tile([128, NB], FP32, tag="t1")
    nc.vector.tensor_tensor(t1[:], p_v[:], A_v[:], ALU.mult)
    t2 = vec_p.tile([128, NB], FP32, tag="t2")
    nc.vector.tensor_tensor(t2[:], q_v[:], B_v[:], ALU.mult)
    D_v = vec_p.tile([128, NB], FP32, tag="D_v")
    nc.vector.tensor_tensor(D_v[:], t1[:], t2[:], ALU.add)
    warm_mm(4)
    invD = vec_p.tile([128, NB], FP32, tag="invD")
    nc.vector.reciprocal(invD[:], D_v[:])
    r_v = vec_p.tile([128, NB], FP32, tag="r_v")
    nc.vector.tensor_tensor(r_v[:], p_v[:], invD[:], ALU.mult)
    u_v = vec_p.tile([128, NB], FP32, tag="u_v")
    nc.vector.tensor_tensor(u_v[:], q_v[:], invD[:], ALU.mult)
    nc.vector.tensor_copy(Rk[:, :, 0], r_v[:])
    r_hi32 = vec_p.tile([128, NB], FP32, tag="r_hi32")
    nc.vector.tensor_copy(r_hi32[:], Rk[:, :, 0])
    nc.vector.tensor_tensor(Rk[:, :, 1], r_v[:], r_hi32[:], ALU.subtract)
    nc.vector.tensor_scalar(Rk[:, :, 2], u_v[:], -1.0, None, ALU.mult)
    un_hi32 = vec_p.tile([128, NB], FP32, tag="un_hi32")
    nc.vector.tensor_copy(un_hi32[:], Rk[:, :, 2])
    nc.vector.scalar_tensor_tensor(
        Rk[:, :, 3], un_hi32[:], -1.0, u_v[:], ALU.mult, ALU.subtract
    )
    nc.vector.tensor_scalar(Rkh[:, :, 0:4], Rk[:, :, 0:4], 0.5, None, ALU.mult)
    warm_mm(5)

    # Rtot/Utot -> half-total broadcast htot2 [128, 2]
    ru_s = vec_p.tile([128, 2], FP32, tag="ru_s")
    nc.vector.reduce_sum(ru_s[:, 0:1], r_v[:], axis=mybir.AxisListType.X)
    nc.vector.reduce_sum(ru_s[:, 1:2], u_v[:], axis=mybir.AxisListType.X)
    tot2_ps = ps_sm.tile([1, 2], FP32, tag="sm")
    nc.tensor.matmul(tot2_ps[:], lhsT=ones_f[:], rhs=ru_s[:], start=True, stop=True)
    tot2_sb = vec_p.tile([1, 2], FP32, tag="tot2_sb")
    nc.vector.tensor_copy(tot2_sb[:], tot2_ps[:])
    htot2_ps = ps_sm.tile([128, 2], FP32, tag="sm")
    nc.tensor.matmul(
        htot2_ps[:], lhsT=halves_row_f[:], rhs=tot2_sb[:], start=True, stop=True
    )
    htot2 = vec_p.tile([128, 2], FP32, tag="htot2")
    nc.vector.tensor_copy(htot2[:], htot2_ps[:])

    # ---------------- lrl transposes [tok, f], just-in-time per chunk -----
    lrl_sb = big_p.tile([128, NB, 128], FP32, tag="lrl_sb")

    def lrl_chunk(c):
        for h in range(2):
            t2 = 2 * c + h
            op = ps_tr.tile([128, 256], FP32, tag="tr")
            nc.tensor.matmul(
                op[:, 0:128], lhsT=lrlT[:, 256 * t2 : 256 * t2 + 128], rhs=ident_f[:],
                is_transpose=True, start=True, stop=False,
            )
            nc.tensor.matmul(
                op[:, 128:256], lhsT=lrlT[:, 256 * t2 + 128 : 256 * t2 + 256],
                rhs=ident_f[:],
                is_transpose=True, start=False, stop=True,
            )
            if h == 0:
                nc.scalar.copy(lrl_sb[:, 2 * t2 : 2 * t2 + 2, :], op[:])
            else:
                nc.vector.tensor_copy(lrl_sb[:, 2 * t2 : 2 * t2 + 2, :], op[:])

    for _c in range(NC4):
        lrl_chunk(_c)
        if _c % 2 == 1:
            warm_mm(6 + _c)

    # ---------------- matvec 2 (c-outer, col-tiled, pipelined tails) ------
    out_view = out_d.rearrange("(t p) f -> p t f", p=128)
    g_ps = ps_big.tile([128, N], FP32, tag="bigps")
    Gt2_ps = ps_sm.tile([128, NB, 2], FP32, tag="sm")
    gsb = vec_p.tile([128, NB, 2], FP32, tag="gsb")
    gA = vec_p.tile([128, NB], FP32, tag="gA")
    gB = vec_p.tile([128, NB], FP32, tag="gB")
    gt1 = vec_p.tile([128, NB], FP32, tag="gt1")
    col = vec_p.tile([128, NB], FP32, tag="col")

    def mv2_chunk(c):
        for rr in range(NC4):
            for j in range(4):
                a = 4 * rr + j
                nc.tensor.matmul(
                    g_ps[32 * j : 32 * j + 32, c * 512 : (c + 1) * 512],
                    lhsT=stat_for(Rk, Rkh, a),
                    rhs=mask_tiles[a][:, c * 512 : (c + 1) * 512],
                    start=(rr == 0),
                    stop=(rr == 3),
                    tile_position=(0, 32 * j),
                    skip_group_check=True,
                )

    def mv2_tail(c):
        mv_tail_chunk(g_ps, Gt2_ps, c, nc.scalar.copy)
        ts = slice(c * 4, (c + 1) * 4)
        nc.vector.tensor_copy(gsb[:, ts, :], Gt2_ps[:, ts, :])
        nc.vector.tensor_scalar(gA[:, ts], gsb[:, ts, 0], htot2[:, 0:1], None, ALU.add)
        nc.vector.tensor_scalar(gB[:, ts], gsb[:, ts, 1], htot2[:, 1:2], None, ALU.add)
        nc.vector.tensor_tensor(gt1[:, ts], p_v[:, ts], gA[:, ts], ALU.mult)
        nc.vector.tensor_tensor(col[:, ts], q_v[:, ts], gB[:, ts], ALU.mult)
        nc.vector.tensor_tensor(col[:, ts], col[:, ts], gt1[:, ts], ALU.add)
        for tt in range(4):
            t = c * 4 + tt
            o_sb = outsb_p.tile([128, 128], FP32, tag="o_sb")
            nc.vector.tensor_scalar(
                o_sb[:], lrl_sb[:, t, :], col[:, t : t + 1], None, ALU.mult
            )
            dma_eng[0 if tt % 2 == 0 else 2].dma_start(out_view[:, t, :], o_sb[:])

    mv2_chunk(0)
    mv2_chunk(1)
    mv2_tail(0)
    mv2_chunk(2)
    mv2_tail(1)
    mv2_chunk(3)
    mv2_tail(2)
    mv2_tail(3)


def build_nc(num_devices: int = 8) -> "bass.Bass":
    nc = bacc.Bacc(
        "TRN2", target_bir_lowering=False, debug=False, num_devices=num_devices
    )
    x_d = nc.dram_tensor("x", [N, F], FP32, kind="ExternalInput")
    W_d = nc.dram_tensor("W", [F, F], FP32, kind="ExternalInput")
    wm_d = nc.dram_tensor("w_mlp", [F], FP32, kind="ExternalInput")
    bm_d = nc.dram_tensor("b_mlp", [1], FP32, kind="ExternalInput")
    out_d = nc.dram_tensor("out", [N, F], FP32, kind="ExternalOutput")
    with tile.TileContext(nc) as tc:
        with ExitStack() as ctx:
            gat_kernel(ctx, tc, out_d.ap(), x_d.ap(), W_d.ap(), wm_d.ap(), bm_d.ap())
    nc.compile()
    return nc


_NC_CACHE: dict = {}


def run(x, W, w_mlp, b_mlp, trace=False, **spmd_kwargs):
    x = np.asarray(x, dtype=np.float32)
    W = np.asarray(W, dtype=np.float32)
    w_mlp = np.asarray(w_mlp, dtype=np.float32)
    b_mlp = np.asarray(b_mlp, dtype=np.float32)

    if "nc" not in _NC_CACHE:
        _NC_CACHE["nc"] = build_nc(num_devices=B)
    nc = _NC_CACHE["nc"]

    in_maps = [
        {"x": np.ascontiguousarray(x[b, 0]), "W": W, "w_mlp": w_mlp, "b_mlp": b_mlp}
        for b in range(B)
    ]
    res = run_bass_kernel_spmd(
        nc, in_maps, core_ids=list(range(B)), trace=trace, **spmd_kwargs
    )
    out = np.stack([res.results[b]["out"] for b in range(B)])[:, None]
    return out.astype(np.float32), res


def kernel(x, W, w_mlp, b_mlp):
    out, _ = run(x, W, w_mlp, b_mlp)
    return out
